# revision 8
# baseline (speedup 1.0000x reference)
"""Center loss kernel for Trainium2, 8 NeuronCores, data-parallel over batch.

loss = sum((x - centers[labels])**2) / 2 / BATCH

v6 (primary): expand the square,

    sum((x-c)^2) = sum(x^2) - 2*sum_s S_s.c_s + sum_u count_u*|c_u|^2

The batch is sorted by label on the host (the loss is permutation
invariant), sharded 8 ways, and x is quantized to fp8 e4m3 on the host
(4 MB per core instead of 16 MB; the quantization bias on the loss is
~7e-4 relative, far inside the 2e-2 gate). Per 128-row tile the sorted
labels touch <=16 distinct centers, so tile t maps its rows to 16 slots
of a 128-slot PSUM group (8 tiles per group, 2 groups). The idle
TensorEngine turns the row->slot one-hots into class sums S (fp8
matmuls accumulating in PSUM), and the cross term becomes two
[128,2048] dot-reduces against host-pregathered fp8 centers. The only
heavy elementwise work left is sum(x̂^2): 16 tile-squares split across
ScalarE (activation Square + accum), VectorE (fused tensor_tensor_reduce)
and GpSimd (mult + partition-reduce). The count_u*|c_u|^2 term depends
only on labels/centers and is summed on the host in f64.

x̂ is laid out on the host as [128, 16*2048] (tile-major columns) so each
of the 8 chunk DMAs moves 128 contiguous 4 KB descriptors; chunks are
spread across the sync/vector/scalar HWDGE queues so descriptor issue
never starves the 16 DMA engines.

Fallback (v2, any label distribution): per-row indirect gather of bf16
centers straight from DRAM.
"""

import numpy as np
import ml_dtypes

import concourse.bacc as bacc
import concourse.bass as bass
import concourse.mybir as mybir
import concourse.tile as tile
from concourse.bass_utils import run_bass_kernel_spmd
from concourse.dve_ops import AFFINE_MUL_REDUCE

N_CORES = 8
BATCH = 16384
FEAT = 2048
NUM_CLASSES = 1000
B_SHARD = BATCH // N_CORES  # 2048
P = 128
N_TILES = B_SHARD // P  # 16
U = 16  # max distinct centers per 128-row tile (sorted labels)
N_CHUNKS = 8  # 2 tiles per DMA chunk
FP8 = ml_dtypes.float8_e4m3

# chunk -> square engine: 'a' scalar, 'd' vector, 'g' gpsimd, 's' = split a/d
CHUNK_ENG = ["a", "d", "a", "d", "a", "g", "a", "s"]


def _build_v6():
    nc = bacc.Bacc("TRN2", num_devices=N_CORES)
    xq = nc.dram_tensor(
        "xq", [P, N_TILES * FEAT], mybir.dt.float8e4, kind="ExternalInput"
    ).ap()
    oh = nc.dram_tensor(
        "oh", [P, N_TILES * P], mybir.dt.float8e4, kind="ExternalInput"
    ).ap()
    cg = nc.dram_tensor(
        "cg", [P, 2 * FEAT], mybir.dt.float8e4, kind="ExternalInput"
    ).ap()
    out = nc.dram_tensor("out", [P, 10], mybir.dt.float32, kind="ExternalOutput").ap()
    outg = nc.dram_tensor(
        "outg", [1, 2 * FEAT], mybir.dt.float32, kind="ExternalOutput"
    ).ap()

    with tile.TileContext(nc) as tc:
        with (
            tc.tile_pool(name="sbuf", bufs=4) as pool,
            tc.tile_pool(name="persist", bufs=1) as ppool,
            tc.tile_pool(name="psum", bufs=1, space="PSUM") as psum_pool,
        ):
            # [row, tile, slot] one-hots: lhsT pair slices for DoubleRow
            oh_s = ppool.tile([P, 2 * N_CHUNKS, P], mybir.dt.float8e4)
            nc.gpsimd.dma_start(out=oh_s[:], in_=oh)
            cg_s = ppool.tile([P, 2 * FEAT], mybir.dt.float8e4)
            nc.sync.dma_start(out=cg_s[:], in_=cg)

            acc = ppool.tile([P, 10], mybir.dt.float32)
            sc_a = ppool.tile([P, 2, FEAT], mybir.dt.bfloat16)  # act square out
            sc_d = ppool.tile([P, 2, FEAT], mybir.dt.bfloat16)  # dve square out
            sc_g = ppool.tile([P, 2, FEAT], mybir.dt.bfloat16)  # gp mult out
            gsum = ppool.tile([1, 2, FEAT], mybir.dt.float32)

            psum_g = [
                psum_pool.tile([P, FEAT], mybir.dt.float32, tag="g0", name="psg0"),
                psum_pool.tile([P, FEAT], mybir.dt.float32, tag="g1", name="psg1"),
            ]

            q_of = {
                0: nc.sync,
                1: nc.scalar,
                2: nc.sync,
                3: nc.scalar,
                4: nc.sync,
                5: nc.scalar,
                6: nc.sync,
                7: nc.gpsimd,
            }
            na = nd = 0
            for c in range(N_CHUNKS):
                xt = pool.tile([P, 2, FEAT], mybir.dt.float8e4, tag="xc")
                q_of[c].dma_start(
                    out=xt[:], in_=xq[:, c * 2 * FEAT : (c + 1) * 2 * FEAT]
                )
                g = c // 4
                for j in range(FEAT // 512):
                    js = slice(j * 512, (j + 1) * 512)
                    nc.tensor.matmul(
                        out=psum_g[g][:, js],
                        lhsT=oh_s[:, 2 * c : 2 * c + 2, :],
                        rhs=xt[:, :, js],
                        start=(c % 4 == 0),
                        stop=(c % 4 == 3),
                        perf_mode=mybir.MatmulPerfMode.DoubleRow,
                    )
                eng = CHUNK_ENG[c]
                if eng == "a":
                    nc.scalar.activation(
                        out=sc_a[:],
                        in_=xt[:],
                        func=mybir.ActivationFunctionType.Square,
                        accum_out=acc[:, na : na + 1],
                    )
                    na += 1
                elif eng == "d":
                    nc.vector._custom_dve(
                        AFFINE_MUL_REDUCE,
                        out=sc_d[:],
                        in0=xt[:],
                        in1=xt[:],
                        s0=1.0,
                        s1=0.0,
                        accum_out=acc[:, 5 + nd : 6 + nd],
                    )
                    nd += 1
                elif eng == "g":
                    nc.gpsimd.tensor_tensor(
                        out=sc_g[:], in0=xt[:], in1=xt[:], op=mybir.AluOpType.mult
                    )
                    nc.gpsimd.tensor_reduce(
                        out=gsum[:],
                        in_=sc_g[:],
                        axis=mybir.AxisListType.C,
                        op=mybir.AluOpType.add,
                    )
                else:  # split: act half 0, dve half 1
                    nc.scalar.activation(
                        out=sc_a[:, 0, :],
                        in_=xt[:, 0, :],
                        func=mybir.ActivationFunctionType.Square,
                        accum_out=acc[:, na : na + 1],
                    )
                    na += 1
                    nc.vector._custom_dve(
                        AFFINE_MUL_REDUCE,
                        out=sc_d[:, 1, :],
                        in0=xt[:, 1, :],
                        in1=xt[:, 1, :],
                        s0=1.0,
                        s1=0.0,
                        accum_out=acc[:, 5 + nd : 6 + nd],
                    )
                    nd += 1
                if c == 3 or c == 7:
                    # cross term for the finished PSUM group, -2 folded in
                    nc.vector._custom_dve(
                        AFFINE_MUL_REDUCE,
                        out=sc_d[:, 0, :],
                        in0=psum_g[g][:],
                        in1=cg_s[:, g * FEAT : (g + 1) * FEAT],
                        s0=-2.0,
                        s1=0.0,
                        accum_out=acc[:, 8 + g : 9 + g],
                    )
            nc.scalar.dma_start(out=out, in_=acc[:])
            nc.gpsimd.dma_start(out=outg, in_=gsum[:])
    nc.finalize()
    return nc


def _build_v2():
    nc = bacc.Bacc("TRN2", num_devices=N_CORES)
    x = nc.dram_tensor("x", [B_SHARD, FEAT], mybir.dt.float32, kind="ExternalInput").ap()
    labels = nc.dram_tensor(
        "labels", [P, N_TILES], mybir.dt.int32, kind="ExternalInput"
    ).ap()
    cb = nc.dram_tensor(
        "cb", [NUM_CLASSES, FEAT], mybir.dt.bfloat16, kind="ExternalInput"
    ).ap()
    out = nc.dram_tensor("out", [P, 1], mybir.dt.float32, kind="ExternalOutput").ap()

    with tile.TileContext(nc) as tc:
        with (
            tc.tile_pool(name="sbuf", bufs=3) as pool,
            tc.tile_pool(name="persist", bufs=1) as ppool,
        ):
            lab = ppool.tile([P, N_TILES], mybir.dt.int32)
            nc.sync.dma_start(out=lab[:], in_=labels)
            acc = ppool.tile([P, N_TILES], mybir.dt.float32)
            for t in range(N_TILES):
                xt = pool.tile([P, FEAT], mybir.dt.float32, tag="xt")
                nc.sync.dma_start(out=xt[:], in_=x[t * P : (t + 1) * P, :])
                g = pool.tile([P, FEAT], mybir.dt.bfloat16, tag="g")
                nc.gpsimd.indirect_dma_start(
                    out=g[:],
                    out_offset=None,
                    in_=cb,
                    in_offset=bass.IndirectOffsetOnAxis(ap=lab[:, t : t + 1], axis=0),
                )
                d = pool.tile([P, FEAT], mybir.dt.float32, tag="d")
                nc.vector.tensor_tensor(
                    out=d[:], in0=xt[:], in1=g[:], op=mybir.AluOpType.subtract
                )
                nc.scalar.activation(
                    out=d[:],
                    in_=d[:],
                    func=mybir.ActivationFunctionType.Square,
                    accum_out=acc[:, t : t + 1],
                )
            accp = ppool.tile([P, 1], mybir.dt.float32)
            nc.vector.tensor_reduce(
                out=accp[:], in_=acc[:], axis=mybir.AxisListType.X, op=mybir.AluOpType.add
            )
            nc.sync.dma_start(out=out, in_=accp[:])
    nc.finalize()
    return nc


_CACHE = {}


def _prep_v6(x, labels_i, centers):
    """Sort by label, shard, quantize to fp8, build one-hot/slot metadata.

    Returns (in_maps, c2_total) or None if some tile needs more than U
    distinct centers.
    """
    order = np.argsort(labels_i, kind="stable")
    labs = labels_i[order]
    xs_all = x[order]
    cq = centers.astype(FP8)

    counts = np.bincount(labels_i.astype(np.int64), minlength=NUM_CLASSES)
    c2 = (centers.astype(np.float64) ** 2).sum(axis=1)
    c2_total = float((counts * c2).sum())

    in_maps = []
    rows = np.arange(P)
    for c in range(N_CORES):
        sl = slice(c * B_SHARD, (c + 1) * B_SHARD)
        ls = labs[sl]
        oh = np.zeros((P, N_TILES * P), dtype=FP8)
        cgidx = np.zeros((2, P), dtype=np.int64)
        for t in range(N_TILES):
            lt = ls[t * P : (t + 1) * P]
            uu, inv = np.unique(lt, return_inverse=True)
            if len(uu) > U:
                return None
            g, base = t // 8, U * (t % 8)
            oh[rows, t * P + base + inv] = 1.0
            cgidx[g, base : base + len(uu)] = uu
        cg = np.concatenate([cq[cgidx[0]], cq[cgidx[1]]], axis=1)
        xq = (
            xs_all[sl]
            .astype(FP8)
            .reshape(N_TILES, P, FEAT)
            .transpose(1, 0, 2)
            .reshape(P, N_TILES * FEAT)
        )
        in_maps.append(
            {
                "xq": np.ascontiguousarray(xq),
                "oh": oh,
                "cg": np.ascontiguousarray(cg),
            }
        )
    return in_maps, c2_total


def _prep_v2(x, labels_i, centers):
    cb = centers.astype(ml_dtypes.bfloat16)
    in_maps = []
    for c in range(N_CORES):
        sl = slice(c * B_SHARD, (c + 1) * B_SHARD)
        lab = np.ascontiguousarray(
            labels_i[sl].astype(np.int32).reshape(N_TILES, P).T
        )
        in_maps.append({"x": np.ascontiguousarray(x[sl]), "labels": lab, "cb": cb})
    return in_maps


def _run(x, labels, centers, trace=False, force=None):
    x = np.ascontiguousarray(np.asarray(x), dtype=np.float32)
    labels_i = np.ascontiguousarray(np.asarray(labels)).astype(np.int64)
    centers = np.ascontiguousarray(np.asarray(centers), dtype=np.float32)
    assert x.shape == (BATCH, FEAT), x.shape
    assert labels_i.shape == (BATCH,), labels_i.shape
    assert centers.shape == (NUM_CLASSES, FEAT), centers.shape

    in_maps = None
    c2_total = 0.0
    variant = force or "v6"
    if variant == "v6":
        prep = _prep_v6(x, labels_i, centers)
        if prep is None:
            variant = "v2"
        else:
            in_maps, c2_total = prep
    if variant == "v2":
        in_maps = _prep_v2(x, labels_i, centers)

    if variant not in _CACHE:
        _CACHE[variant] = _build_v6() if variant == "v6" else _build_v2()
    nc = _CACHE[variant]

    res = run_bass_kernel_spmd(nc, in_maps, core_ids=list(range(N_CORES)), trace=trace)
    total = 0.0
    if variant == "v6":
        for c in range(N_CORES):
            total += float(res.results[c]["out"].astype(np.float64).sum())
            total += float(res.results[c]["outg"].astype(np.float64).sum())
        total += c2_total
    else:
        for c in range(N_CORES):
            total += float(res.results[c]["out"].astype(np.float64).sum())
    val = np.float32(total / 2.0 / BATCH)
    return val, res


def kernel(x, labels, centers):
    val, _ = _run(x, labels, centers)
    return val
